# revision 75
# baseline (speedup 1.0000x reference)
"""Paged-attention GQA decode kernel for Trainium2 (8 NeuronCores).

Problem: B=32 sequences, one new token each; KV cache [65536, 8, 128] f32
paged with PAGE=16; 32 query heads, 8 KV heads (GQA group 4), D=128.

Sharding: each core owns 2 KV heads x 16 sequences (batch split in two
interleaved-by-length halves for balance). Per core there are 32 "units"
(seq, head), each contributing 4 query rows -> 128 partition rows.

The kernel is DMA-bound (must stream the whole valid KV context), so the
packed K is fp8e3 (e3m4: randn fits the +-15.5 range, ~3% rel err, fed
straight into the bf16xfp8 matmul) and V is int8 with one global scale
(multiplied back into the host-side unpack). V streams as 1 byte/elem
split across both HWDGE rings AND the SWDGE queue (each HWDGE ring's
descriptor generator tops out around 145 GB/s, so a third queue is
needed to reach the HBM roofline), then DVE/ACT dequantize int8->bf16
in SBUF ahead of mm2. q is bf16. rel-err ~1.4e-2 (gate 2e-2).

K is packed BAND-MAJOR (all units' positions [512b, 512b+512) before
band b+1) and streamed BEFORE V: each scores PSUM bank closes as soon
as its band has landed, so softmax/transpose/mm2 overlap the V stream
and the only work serialized after the last byte is the final V
group's mm2s (the shortest sequences).

Device pipeline per core:
  qdiag: memset [128, 4224] + one strided DVE copy turns the compact
        [128, 128] q tile into the staggered block-diagonal layout
        (unit u's 4 q columns at cols 132u..132u+4, so the lhsT slice
        [128u, 128u+128) has them at local cols 4u..4u+4).
  mm1:  scores[4u+g, j] = q_u . K_u[j] -- block-diagonal accumulation of
        bf16xfp8 matmuls (one N<=512 matmul per unit per PSUM bank) into
        per-bank scores PSUM tiles; K groups ride both HWDGE rings.
  softmax: exp (ACT, scale=1/sqrt(D), accum_out -> per-bank row sums)
        one bank-wide call, writes bf16; pad columns hold K=0 so exp=1
        there and the host subtracts the pad count from the sum.
  pT:   PE transposes of the bf16 exp tile (chunks of 128 positions).
  mm2:  o_u += pT_chunk(4 cols, stationary) @ V_chunk[128, 128] bf16,
        waves per V group, round-robin over PE column strips.
  host: o = o_rows / (sum - npad), un-permute.

The program is value-specialized on the sorted per-unit chunk counts
(from context_lens); compiled programs are cached per signature.
"""

import numpy as np
import ml_dtypes

B = 32
KV_LEN = 2048
PAGE = 16
PAGES = KV_LEN // PAGE
H_Q = 32
H_KV = 8
D = 128
CH = 128                 # slot chunk (matmul contraction tile)
NCORES = 8
P = 128
SPC = 16                 # seqs per core
UPC = 32                 # units (seq, head) per core
SCALE = np.float32(1.0 / np.sqrt(D))
QW = 132                 # staggered block-diag column period for qT

_PROGRAM_CACHE = {}


def _k_groups(vps, max_cols):
    """Greedy-pack consecutive units into DMA groups of <=max_cols."""
    groups, cur, cols = [], [], 0
    for i, c in enumerate(vps):
        if cur and cols + c > max_cols:
            groups.append(cur)
            cur, cols = [], 0
        cur.append(i)
        cols += c
    if cur:
        groups.append(cur)
    return groups


def _k_layout(ntU, gcols):
    """Band-major K layout. Returns (ksegs, koff, kgroups) where ksegs is
    a list of (band, unit, width) column segments in stream order, koff
    the per-segment column offsets, and kgroups greedy packs of
    consecutive segment indices (<= gcols columns each, never spanning a
    band boundary so each score bank closes at its own last byte)."""
    vpU = [ntU[u // 2] * CH for u in range(UPC)]
    nbanks = (ntU[0] + 3) // 4
    ksegs = []
    kgroups = []
    for b in range(nbanks):
        first = len(ksegs)
        for u in range(UPC):
            w = min(512, vpU[u] - 512 * b)
            if w > 0:
                ksegs.append((b, u, w))
        for grp in _k_groups([w for _, _, w in ksegs[first:]], gcols):
            kgroups.append([first + si for si in grp])
    koff = np.zeros(len(ksegs) + 1, np.int64)
    koff[1:] = np.cumsum([w for _, _, w in ksegs])
    return ksegs, koff, kgroups


def _build_program(ntU):
    """Build + compile the per-core program. ntU = per-seq-slot chunk counts
    (descending, len 16); unit u=2s+h has ntU[s] chunks."""
    import concourse.bacc as bacc
    import concourse.mybir as mybir
    import concourse.tile as tile
    from concourse.masks import make_identity
    from concourse.tile import add_dep_helper

    f32 = mybir.dt.float32
    bf16 = mybir.dt.bfloat16
    f8 = mybir.dt.float8e3
    i8 = mybir.dt.int8
    GCOLS = 8192             # dma group size (columns)

    vpU = [ntU[u // 2] * CH for u in range(UPC)]          # per-unit K cols
    TOTK = int(sum(vpU))
    v_cols = [ntU[s] * 2 * CH for s in range(SPC)]         # per-seq V cols
    v_off = np.zeros(SPC + 1, np.int64)
    v_off[1:] = np.cumsum(v_cols)
    assert int(v_off[-1]) == TOTK
    maxnt = ntU[0]
    nbanks = (maxnt + 3) // 4

    ksegs, koff, kgroups = _k_layout(ntU, GCOLS)
    vgroups = _k_groups(v_cols, GCOLS)
    # PSUM bank = 512 f32 cols = 4 chunks. Per bank: its last writer unit
    # (stop there -- start/stop have whole-bank HW side effects, so unit 0
    # opens each bank with its band-leading matmul and the stop rides the
    # final write of the bank). mm1 is one matmul per (unit, bank).
    bank_stop = {}
    for b in range(nbanks):
        bank_stop[b] = max(2 * s + 1 for s in range(SPC) if ntU[s] > 4 * b)
    # kgroup that closes bank b (contains the last segment of band b)
    seg_kg = {si: gi for gi, grp in enumerate(kgroups) for si in grp}
    bank_close_kg = {}
    for si, (b, u, w) in enumerate(ksegs):
        bank_close_kg[b] = seg_kg[si]
    # V group of each slot
    slot_vg = {s: gi for gi, grp in enumerate(vgroups) for s in grp}

    nc = bacc.Bacc(
        "TRN2",
        target_bir_lowering=False,
        debug=False,
        enable_asserts=False,
        num_devices=NCORES,
    )
    qc_d = nc.dram_tensor("qc", [P, UPC * 4], bf16, kind="ExternalInput").ap()
    kT_d = nc.dram_tensor("kT", [P, TOTK], f8, kind="ExternalInput").ap()
    vt_d = nc.dram_tensor("vt", [P, TOTK], i8, kind="ExternalInput").ap()
    o_d = nc.dram_tensor("o", [P, 1024], bf16, kind="ExternalOutput").ap()
    s_d = nc.dram_tensor("s", [P, nbanks], f32, kind="ExternalOutput").ap()

    with tile.TileContext(nc) as tc:
        with (
            tc.tile_pool(name="const", bufs=1) as constp,
            tc.tile_pool(name="kpool", bufs=len(kgroups)) as kpool,
            tc.tile_pool(name="vpool", bufs=len(vgroups)) as vpool,
            tc.tile_pool(name="vbpool", bufs=len(vgroups)) as vbpool,
            tc.tile_pool(name="sm", bufs=1) as smp,
            tc.tile_pool(name="tp", bufs=2, space="PSUM") as tpp,
            tc.tile_pool(name="ps_scores", bufs=1, space="PSUM") as pssc,
            tc.tile_pool(name="ps_o", bufs=1, space="PSUM") as pso,
        ):
            # Compact q load (32 KB) + on-device expansion to the staggered
            # block-diagonal stationary layout. The load leads the scalar
            # ring at priority 0 -- everything gates on qT.
            qc_s = constp.tile([P, UPC * 4], bf16)
            with tc.high_priority():
                nc.scalar.dma_start(qc_s[:], qc_d[:])
            qT_s = constp.tile([P, QW * UPC], bf16)
            nc.vector.memset(qT_s[:], 0.0)
            qT3 = qT_s[:].rearrange("p (u c) -> p u c", u=UPC)
            qc3 = qc_s[:].rearrange("p (u g) -> p u g", u=UPC)
            nc.vector.tensor_copy(qT3[:, :, 0:4], qc3[:, :, :])
            ident = constp.tile([P, P], bf16)
            make_identity(nc, ident[:])
            zt = constp.tile([P, 512], bf16)
            nc.vector.memset(zt[:], 0.0)

            s_parts = smp.tile([P, nbanks], f32)
            # per-bank scores tiles: separate dependency domains, so exp of
            # a high bank unblocks as soon as that bank's writers finish
            scores_b = [pssc.tile([P, 512], f32, space="PSUM", tag=f"sc{b}",
                                  name=f"scores{b}")
                        for b in range(nbanks)]
            # 32 concurrent [4, 128] output accumulators packed into TWO
            # psum banks at 32-aligned partition blocks (matmul out base
            # partition must be a multiple of 32): unit u lives in bank
            # u//16, partitions 32*(u%4).., cols 128*((u//4)%4)..
            obank = [pso.tile([P, 512], f32, space="PSUM", name=f"ob{i}")
                     for i in range(2)]

            # ---- DMA issue, all upfront. K (band-major, fp8) alternates
            # on the two HWDGE rings; V (int8) is split between the SWDGE
            # (gpsimd) queue (even groups) and whichever HWDGE ring is
            # lighter (odd groups) -- each HWDGE descriptor generator tops
            # out near 145 GB/s, so three queues are needed to reach the
            # HBM roofline. Everything is SBUF-resident so nothing
            # downstream ever blocks the rings.
            # ---- DMA issue, all upfront: K and V groups alternate on both
            # HWDGE rings (K slightly ahead) exactly in the order the
            # consumers need them -- emission order, arrival order, and
            # compute order all line up, so the scheduler's frozen
            # per-engine streams match real data arrival.
            kts, vts, vbs = [], [], []
            rings = [nc.sync, nc.scalar]
            # byte-progress interleave: keep the K stream ~one group ahead
            # of V so bands close just before their dependents need them
            ki = vi = 0
            kb = vb_ = 0
            ring_i = 0
            while ki < len(kgroups) or vi < len(vgroups):
                pick_k = ki < len(kgroups) and (
                    vi >= len(vgroups) or kb <= vb_ + GCOLS)
                if pick_k:
                    grp = kgroups[ki]
                    g0, g1 = int(koff[grp[0]]), int(koff[grp[-1] + 1])
                    kt = kpool.tile([P, GCOLS], f8, tag="k", name=f"kt{ki}")
                    rings[ring_i].dma_start(kt[:, :g1 - g0], kT_d[:, g0:g1])
                    kts.append(kt)
                    kb += g1 - g0
                    ki += 1
                else:
                    grp = vgroups[vi]
                    g0, g1 = int(v_off[grp[0]]), int(v_off[grp[-1] + 1])
                    vt = vpool.tile([P, GCOLS], i8, tag="v", name=f"vt{vi}")
                    rings[ring_i].dma_start(vt[:, :g1 - g0], vt_d[:, g0:g1])
                    vts.append(vt)
                    vb_ += g1 - g0
                    vi += 1
                ring_i = 1 - ring_i

            # open each accumulator bank once with a bank-wide zero matmul
            # (start clears has_written bank-wide, so it must happen exactly
            # once per bank); every mm2 then accumulates (start=False) in
            # any order
            zmm = [nc.tensor.matmul(obank[i][:, :], ident[:], zt[:],
                                    start=True, stop=False,
                                    skip_group_check=True)
                   for i in range(2)]

            # ---- interleaved emission: per K group, mm1 -> newly closed
            # banks' exp/transpose/pT -> mm2 wave over (unit, chunk) pairs
            # whose pT is closed and whose V group has (heuristically)
            # landed. The PE stream is in-order, so waves only include work
            # whose data should be resident by then.
            e_c, pT_c = {}, {}
            emitted = set()
            n_mm2 = sum(2 * ntU[s] for s in range(SPC))
            # per-obank mm2 counts so each accumulation group gets its own
            # stop and the bank can be copied out as soon as it completes
            n_bank = [sum(2 * ntU[s] for s in range(8)),
                      sum(2 * ntU[s] for s in range(8, SPC))]
            cnt_bank = [0, 0]
            closed = [False] * nbanks

            exp_ins = {}

            def emit_softmax(b):
                # one bank-wide exp (ACT) + per-chunk PE transposes; the
                # whole chain runs at priority 0 so the scheduler never
                # parks it behind a multi-us dequant cast slice
                with tc.high_priority():
                    w = (min(4 * b + 4, maxnt) - 4 * b) * CH
                    e_c[b] = smp.tile([P, 512], bf16, tag=f"e{b}",
                                      name=f"e{b}")
                    exp_ins[b] = nc.scalar.activation(
                        e_c[b][:, :w], scores_b[b][:, :w],
                        mybir.ActivationFunctionType.Exp,
                        scale=float(SCALE),
                        accum_out=s_parts[:, b:b + 1])
                    for c in range(min(4 * b + 4, maxnt) - 1, 4 * b - 1, -1):
                        bo = (c % 4) * CH
                        tp = tpp.tile([P, P], bf16, space="PSUM", tag="tp",
                                      name=f"tp{c}")
                        nc.tensor.transpose(tp[:], e_c[b][:, bo:bo + CH],
                                            ident[:])
                        pT_c[c] = smp.tile([P, CH], bf16, tag=f"pT{c}",
                                           name=f"pT{c}")
                        nc.vector.tensor_copy(pT_c[c][:], tp[:])

            def emit_mm2(u, c):
                s, h = u // 2, u % 2
                soff = int(v_off[s]) - int(v_off[vgroups[slot_vg[s]][0]])
                bi, pb, cr = u // 16, 32 * (u % 4), 128 * ((u // 4) % 4)
                cnt_bank[bi] += 1
                mm = nc.tensor.matmul(
                    obank[bi][pb:pb + 4, cr:cr + D],
                    pT_c[c][:, 4 * u:4 * u + 4],
                    vbs[slot_vg[s]][:, soff + (2 * c + h) * D:
                                    soff + (2 * c + h) * D + D],
                    start=False,
                    stop=(cnt_bank[bi] == n_bank[bi]),
                    skip_group_check=True,
                    tile_position=(0, pb),
                )
                add_dep_helper(mm.ins, zmm[bi].ins,
                               reason="o bank opened by zero-matmul first")

            cast_done = set()

            def ensure_cast(vg):
                # int8->bf16 dequant of one V group, split DVE (0.5
                # ns/col) / ACT (0.92 ns/col) by measured rate, two slices
                # per engine so no slice parks an engine queue for
                # multiple us
                if vg in cast_done:
                    return
                cast_done.add(vg)
                g0, g1 = int(v_off[vgroups[vg][0]]), \
                    int(v_off[vgroups[vg][-1] + 1])
                w = g1 - g0
                vb = vbpool.tile([P, GCOLS], bf16, tag="vb", name=f"vb{vg}")
                w1 = (int(w * 0.85) // 256) * 256
                for a, b2 in ((0, w1 // 2), (w1 // 2, w1)):
                    nc.vector.tensor_copy(vb[:, a:b2], vts[vg][:, a:b2])
                for a, b2 in ((w1, (w1 + w) // 2), ((w1 + w) // 2, w)):
                    nc.scalar.copy(vb[:, a:b2], vts[vg][:, a:b2])
                assert len(vbs) == vg
                vbs.append(vb)

            def emit_wave(vg_limit):
                # round-robin the 4 PE column strips (u % 4) so consecutive
                # mm2s land in different col groups and run concurrently;
                # only chunks whose score bank closed and whose V group is
                # dequantized are eligible
                queues = [[] for _ in range(4)]
                for vg in range(min(vg_limit, len(vgroups) - 1) + 1):
                    if vg not in cast_done:
                        continue
                    for s in vgroups[vg]:
                        for h in (0, 1):
                            u = 2 * s + h
                            for c in range(ntU[s] - 1, -1, -1):
                                if (u, c) in emitted or not closed[c // 4]:
                                    continue
                                queues[u % 4].append((u, c))
                                emitted.add((u, c))
                while any(queues):
                    for st in range(4):
                        if queues[st]:
                            emit_mm2(*queues[st].pop(0))
                # flush obank 0 (seqs 0-7) the moment its last mm2 is
                # emitted so its copy+DMA overlap the remaining waves
                if cnt_bank[0] == n_bank[0] and not o_flushed:
                    o_flushed.append(True)
                    nc.vector.tensor_copy(o_sb[:, :512], obank[0][:, :])
                    nc.sync.dma_start(o_d[:, :512], o_sb[:, :512])

            o_sb = smp.tile([P, 1024], bf16)
            o_flushed = []
            first_mm = {}
            for gi, grp in enumerate(kgroups):
                g0 = int(koff[grp[0]])
                kt = kts[gi]
                for si in grp:
                    b, u, w = ksegs[si]
                    soff = int(koff[si]) - g0
                    mm = nc.tensor.matmul(
                        scores_b[b][:, :w],
                        qT_s[:, P * u:P * u + P],
                        kt[:, soff:soff + w],
                        start=(u == 0),
                        stop=(bank_stop[b] == u),
                        skip_group_check=True,
                    )
                    if u == 0:
                        first_mm[b] = mm
                    else:
                        add_dep_helper(
                            mm.ins, first_mm[b].ins,
                            reason="pending-zero: unit-0 mm1 first")
                for b in range(nbanks):
                    if not closed[b] and bank_close_kg[b] == gi:
                        closed[b] = True
                        emit_softmax(b)
                # V group gi-1 has landed by the time K group gi's mm1
                # runs (rings alternate K/V at the same pace); waves trail
                # the casts by one more group so a wave's mm2s never park
                # the in-order PE stream waiting on an in-flight dequant
                for vg in range(min(gi, len(vgroups))):
                    ensure_cast(vg)
                emit_wave(gi - 2)
            assert all(closed)
            for vg in range(len(vgroups)):
                ensure_cast(vg)
            emit_wave(len(vgroups) - 1)
            assert len(emitted) == n_mm2
            # tail: obank 1 via ACT (DVE may still be busy), second o half
            # + sums on the other ring
            nc.scalar.copy(o_sb[:, 512:], obank[1][:, :])
            nc.scalar.dma_start(o_d[:, 512:], o_sb[:, 512:])
            nc.sync.dma_start(s_d[:], s_parts[:])

    nc.compile()
    return nc


def _host_prep(q, k, v, k_cache, v_cache, slot_mapping, block_tables,
               context_lens):
    """Build per-core packed inputs. Returns (in_maps, perm, ntU, sv)."""
    ctx = np.clip(np.asarray(context_lens, np.int64), 1, KV_LEN)
    # global symmetric int8 scale for V (device computes o/sv; the host
    # multiplies sv back in during the unpack)
    sv = max(float(np.abs(v_cache).max()), float(np.abs(v).max())) / 127.0
    nt = (ctx + CH - 1) // CH
    perm = np.argsort(-nt, kind="stable")        # global length-sorted order
    nt_sorted = nt[perm]
    ntU = tuple(int(x) for x in nt_sorted[0::2])  # per-slot padded chunk count

    vpU = [ntU[u // 2] * CH for u in range(UPC)]
    TOTK = int(sum(vpU))
    v_cols = [ntU[s] * 2 * CH for s in range(SPC)]
    v_off = np.zeros(SPC + 1, np.int64)
    v_off[1:] = np.cumsum(v_cols)

    # band-major K segment offsets, keyed by (band, unit)
    ksegs, koff, _ = _k_layout(ntU, 8192)
    seg_off = {(b, u): (int(koff[si]), w)
               for si, (b, u, w) in enumerate(ksegs)}

    bt = np.asarray(block_tables, np.int64)
    ident_bt = np.arange(B * PAGES, dtype=np.int64).reshape(B, PAGES)
    identity_layout = bt.shape == (B, PAGES) and np.array_equal(bt, ident_bt)
    slot_mapping = np.asarray(slot_mapping, np.int64)
    bf16 = ml_dtypes.bfloat16
    fp8 = ml_dtypes.float8_e3m4

    def gather_rows(cache_h, b, cta):
        """rows [0, cta) of seq b's context for one head slice, with the
        new-token scatter applied."""
        if identity_layout:
            rows = cache_h[b * KV_LEN:b * KV_LEN + cta]
            rel = slot_mapping - b * KV_LEN
            hit = np.nonzero((rel >= 0) & (rel < cta))[0]
            patch = (rel[hit], hit) if hit.size else None
        else:
            sids = (bt[b, :, None] * PAGE
                    + np.arange(PAGE)[None, :]).reshape(-1)[:cta]
            rows = cache_h[sids]
            pos, src = np.nonzero(sids[:, None] == slot_mapping[None, :])
            patch = (pos, src) if pos.size else None
        return rows, patch

    in_maps = []
    for m in range(NCORES):
        hb, hp = m // 4, m % 4
        qc = np.zeros((P, UPC * 4), bf16)
        kT_packed = np.zeros((P, TOTK), fp8)
        vt_packed = np.zeros((P, TOTK), np.int8)
        for s in range(SPC):
            b = int(perm[2 * s + hb])
            ntu = ntU[s]
            cta = int(ctx[b])                 # actual cols; rest stays zero
            for h in (0, 1):
                u = 2 * s + h
                head = 2 * hp + h
                qc[:, 4 * u:4 * u + 4] = q[b, 4 * head:4 * head + 4, :].T

                krows, patch = gather_rows(k_cache[:, head, :], b, cta)
                if patch is not None:
                    krows = krows.copy()
                    krows[patch[0]] = k[patch[1], head, :]
                krowsT = krows.T.astype(fp8)
                for band in range((ntu * CH + 511) // 512):
                    o0, w = seg_off[(band, u)]
                    lo = 512 * band
                    hi = min(cta, lo + w)
                    if hi > lo:
                        kT_packed[:, o0:o0 + hi - lo] = krowsT[:, lo:hi]

                vrows, patch = gather_rows(v_cache[:, head, :], b, cta)
                if patch is not None:
                    vrows = vrows.copy()
                    vrows[patch[0]] = v[patch[1], head, :]
                # vt layout per seq: [jj, c, h, d]
                vo = int(v_off[s])
                vt3 = vt_packed[:, vo:vo + ntu * 2 * D].reshape(P, ntu, 2, D)
                vfull = np.zeros((ntu * CH, D), np.float32)
                np.round(vrows / sv, out=vfull[:cta])
                vt3[:, :, h, :] = (vfull.reshape(ntu, CH, D)
                                   .transpose(1, 0, 2).astype(np.int8))

        in_maps.append(dict(qc=qc, kT=kT_packed, vt=vt_packed))

    return in_maps, perm, ntU, sv


def kernel(q, k, v, k_cache, v_cache, slot_mapping, block_tables,
           context_lens, _trace=False):
    from concourse import bass_utils

    q = np.asarray(q, np.float32)
    k = np.asarray(k, np.float32)
    v = np.asarray(v, np.float32)
    k_cache = np.asarray(k_cache, np.float32)
    v_cache = np.asarray(v_cache, np.float32)

    in_maps, perm, ntU, sv = _host_prep(
        q, k, v, k_cache, v_cache, slot_mapping, block_tables, context_lens)

    if ntU not in _PROGRAM_CACHE:
        _PROGRAM_CACHE[ntU] = _build_program(ntU)
    nc = _PROGRAM_CACHE[ntU]

    res = bass_utils.run_bass_kernel_spmd(
        nc, in_maps, core_ids=list(range(NCORES)), trace=_trace)

    ctx = np.clip(np.asarray(context_lens, np.int64), 1, KV_LEN)
    maxpad = ntU[0] * CH
    o = np.empty((B, H_Q, D), np.float32)
    for m in range(NCORES):
        hb, hp = m // 4, m % 4
        om = res.results[m]["o"].astype(np.float32)
        # [128, 1024] bf16: unit u at [32*(u%4)+g,
        #                              512*(u//16) + 128*((u//4)%4) + d]
        ssum = res.results[m]["s"].astype(np.float64).sum(axis=1)
        for s in range(SPC):
            b = int(perm[2 * s + hb])
            denom = ssum[4 * (2 * s):4 * (2 * s) + 8] - (maxpad - int(ctx[b]))
            for h in (0, 1):
                u = 2 * s + h
                head = 2 * hp + h
                pb, co = 32 * (u % 4), 512 * (u // 16) + 128 * ((u // 4) % 4)
                o[b, 4 * head:4 * head + 4, :] = (
                    om[pb:pb + 4, co:co + D] * sv
                    / denom[4 * h:4 * h + 4, None]).astype(np.float32)
    if _trace:
        kernel._last_result = res
    return o

